# revision 17
# baseline (speedup 1.0000x reference)
"""Trainium2 Bass kernel for AGCNODEFunc (gnn_message_passing).

f = tanh(xe + 0.5*a*xa + x@W + x0*sig(beta) - 3x) where
  adj = softmax(relu(emb@emb.T), axis=1); xa = cw*(adj@x)+cb
  S[n,k] = sigmoid(e1[n]e2[k] + bs[n,k]); M = vs@S; Emat = softmax(M, -1); xe = Emat@x
  W = (w*clip(d,0,1))@w.T

Sharding: 8 cores = 4 batches x 2 row-halves. Core c: b=c//2, h=c%2, rows
[h*2048,(h+1)*2048). Fully data-parallel, no collectives. The N^3 matmul
(vs @ S) runs in bf16 on TensorE with flash-style online softmax over k
strips; m is processed in 2 groups of 1024 rows so vs_T(group) + double-
buffered S strips fit SBUF.
"""

import numpy as np
import ml_dtypes

import concourse.bass as bass
import concourse.bacc as bacc
import concourse.mybir as mybir
from concourse import tile, masks
from concourse.bass_utils import run_bass_kernel_spmd

B, N, F, E = 4, 4096, 64, 16
P = 128
MH = N // 2            # 2048 rows per core
NG = 2                 # m-groups per core
MG = MH // NG          # 1024 rows per group
MCH = MG // P          # 8 m-chunks per group
KS = 512               # k-strip width
NSTR = N // KS         # 8 strips per group sweep
NSUB = N // P          # 32 n-subtiles
XT = N // P            # 32 x tiles
f32 = mybir.dt.float32
bf16 = mybir.dt.bfloat16
AF = mybir.ActivationFunctionType
ALU = mybir.AluOpType

_CACHE = {}


def build_nc():
    nc = bacc.Bacc()
    # per-core DRAM parameters
    d_xbT = nc.dram_tensor("x_bT", (F, N), f32, kind="ExternalInput")
    d_xb = nc.dram_tensor("x_b", (N, F), f32, kind="ExternalInput")
    d_xh = nc.dram_tensor("x_h", (MH, F), f32, kind="ExternalInput")
    d_xhT = nc.dram_tensor("x_hT", (F, MH), f32, kind="ExternalInput")
    d_x0 = nc.dram_tensor("x0_h", (MH, F), f32, kind="ExternalInput")
    d_al = nc.dram_tensor("alpha_h", (MH,), f32, kind="ExternalInput")
    d_be = nc.dram_tensor("beta_h", (MH,), f32, kind="ExternalInput")
    d_w12 = nc.dram_tensor("w12", (F, 2), f32, kind="ExternalInput")
    d_wT = nc.dram_tensor("wT", (F, F), f32, kind="ExternalInput")
    d_d = nc.dram_tensor("d", (F,), f32, kind="ExternalInput")
    d_cv = nc.dram_tensor("conv2", (1, 2), f32, kind="ExternalInput")
    d_vsT = nc.dram_tensor("vs_hT", (N, MH), bf16, kind="ExternalInput")
    d_bs = nc.dram_tensor("bs", (N, N), f32, kind="ExternalInput")
    d_embT = nc.dram_tensor("embT", (E, N), bf16, kind="ExternalInput")
    d_embhT = nc.dram_tensor("emb_hT", (E, MH), bf16, kind="ExternalInput")
    d_out = nc.dram_tensor("out", (MH, F), f32, kind="ExternalOutput")

    with tile.TileContext(nc) as tc:
        with (
            tc.tile_pool(name="persist", bufs=1) as persist,
            tc.tile_pool(name="vspool", bufs=1) as vspool,
            tc.tile_pool(name="bsq", bufs=4) as bsqp,
            tc.tile_pool(name="work", bufs=4) as workp,
            tc.tile_pool(name="exp", bufs=3) as expp,
        ):
            ident = persist.tile([P, P], bf16)
            masks.make_identity(nc, ident[:])

            # persistent small tensors
            e2b = persist.tile([P, N], bf16)      # e2 broadcast over partitions
            e12T = persist.tile([P, 2 * NSUB], f32)
            sa = persist.tile([P, MH // P], f32)
            sb_ = persist.tile([P, MH // P], f32)
            cvb = persist.tile([P, 2], f32)
            xe_b = [persist.tile([P, F + 1], bf16, tag=f"xeb{k}", name=f"xeb{k}")
                    for k in range(XT)]
            rest = [persist.tile([P, F], f32, tag=f"rest{m}", name=f"rest{m}")
                    for m in range(MH // P)]
            uacc = [persist.tile([P, F + 1], f32, tag=f"u{m}", name=f"u{m}")
                    for m in range(MH // P)]
            mrun = [persist.tile([P, 1], f32, tag=f"mr{m}", name=f"mr{m}")
                    for m in range(MH // P)]
            lrun = [persist.tile([P, 1], f32, tag=f"lr{m}", name=f"lr{m}")
                    for m in range(MH // P)]
            acc = [persist.tile([P, F], f32, tag=f"acc{m}", name=f"acc{m}")
                   for m in range(MH // P)]
            embT = persist.tile([E, N], bf16)
            embhT = persist.tile([E, MH], bf16)
            nc.sync.dma_start(embT[:], d_embT[:])
            nc.sync.dma_start(embhT[:], d_embhT[:])

            with (
                tc.tile_pool(name="prep", bufs=1) as prep,
                tc.tile_pool(name="xrot", bufs=3) as xrot,
                tc.tile_pool(name="ps_prep", bufs=2, space="PSUM") as ps_prep,
            ):
                # ---------- prep: small parameter math ----------
                wt = prep.tile([F, F], f32)
                nc.sync.dma_start(wt[:], d_wT[:])
                dd = prep.tile([F, 1], f32)
                nc.sync.dma_start(dd[:], d_d[:].rearrange("(f o) -> f o", o=1))
                dcl = prep.tile([F, 1], f32)
                nc.scalar.activation(dcl[:], dd[:], AF.Relu)
                nc.vector.tensor_scalar_min(dcl[:], dcl[:], 1.0)
                wtd = prep.tile([F, F], f32)
                nc.scalar.mul(wtd[:], wt[:], dcl[:, 0:1])
                Wps = ps_prep.tile([P, KS], f32, tag="pp", name="Wps")
                nc.tensor.matmul(Wps[:F, :F], wtd[:], wt[:], start=True, stop=True)
                Wsb = prep.tile([F, F], f32)
                nc.vector.tensor_copy(Wsb[:], Wps[:F, :F])

                w12 = prep.tile([F, 2], f32)
                nc.sync.dma_start(w12[:], d_w12[:])
                xbt = prep.tile([F, N], f32)
                nc.sync.dma_start(xbt[:], d_xbT[:])
                xht = prep.tile([F, MH], f32)
                nc.sync.dma_start(xht[:], d_xhT[:])

                # e2 row form (1, N) on partition 0
                e2row = prep.tile([1, N], f32)
                for c in range(N // 512):
                    eps = ps_prep.tile([P, KS], f32, tag="pp", name="eps")
                    nc.tensor.matmul(eps[:1, :], w12[:, 1:2],
                                     xbt[:, c * 512:(c + 1) * 512],
                                     start=True, stop=True)
                    nc.vector.tensor_copy(e2row[:, c * 512:(c + 1) * 512], eps[:1, :])
                # e12 column form per n-subtile: (128, 2), col0 = e1
                for ns in range(NSUB):
                    eps2 = ps_prep.tile([P, KS], f32, tag="pp", name="eps2")
                    nc.tensor.matmul(eps2[:, :2], xbt[:, ns * P:(ns + 1) * P], w12[:],
                                     start=True, stop=True)
                    nc.vector.tensor_copy(e12T[:, 2 * ns:2 * ns + 2], eps2[:, :2])
                # e2 broadcast to all partitions, bf16
                e2bf = prep.tile([1, N], bf16)
                nc.vector.tensor_copy(e2bf[:], e2row[:])
                nc.gpsimd.partition_broadcast(e2b[:], e2bf[:])

                # alpha/beta sigmoids (128, 16)
                alp = prep.tile([P, MH // P], f32)
                nc.sync.dma_start(alp[:], d_al[:].rearrange("(c p) -> p c", p=P))
                nc.scalar.activation(sa[:], alp[:], AF.Sigmoid)
                bet = prep.tile([P, MH // P], f32)
                nc.sync.dma_start(bet[:], d_be[:].rearrange("(c p) -> p c", p=P))
                nc.scalar.activation(sb_[:], bet[:], AF.Sigmoid)
                cv1 = prep.tile([1, 2], f32)
                nc.sync.dma_start(cv1[:], d_cv[:])
                nc.gpsimd.partition_broadcast(cvb[:], cv1[:])

                # x tiles -> bf16 with ones column (persistent), f32 rotating
                for k in range(XT):
                    xfk = xrot.tile([P, F], f32, tag="xf", name="xf")
                    nc.sync.dma_start(xfk[:], d_xb[k * P:(k + 1) * P, :])
                    nc.scalar.copy(xe_b[k][:, :F], xfk[:])
                    nc.vector.memset(xe_b[k][:, F:F + 1], 1.0)

                # ---------- rest = xw + x0*sig(beta) - 3x ----------
                for m in range(MH // P):
                    xhm = xrot.tile([P, F], f32, tag="xh", name="xh")
                    nc.sync.dma_start(xhm[:], d_xh[m * P:(m + 1) * P, :])
                    x0m = xrot.tile([P, F], f32, tag="x0", name="x0")
                    nc.sync.dma_start(x0m[:], d_x0[m * P:(m + 1) * P, :])
                    xwps = ps_prep.tile([P, KS], f32, tag="pp", name="xwps")
                    nc.tensor.matmul(xwps[:, :F], xht[:, m * P:(m + 1) * P], Wsb[:],
                                     start=True, stop=True)
                    nc.vector.tensor_scalar_mul(rest[m][:], x0m[:], sb_[:, m:m + 1])
                    nc.vector.tensor_tensor(rest[m][:], rest[m][:], xwps[:, :F],
                                            op=ALU.add)
                    tmp3 = workp.tile([P, F], f32, tag="tmp3", name="tmp3")
                    nc.vector.tensor_scalar_mul(tmp3[:], xhm[:], -3.0)
                    nc.vector.tensor_tensor(rest[m][:], rest[m][:], tmp3[:], op=ALU.add)


            # ---------- big loop: M = vs@S, online softmax, xe ----------
            for m in range(MH // P):
                nc.vector.memset(mrun[m][:], -1e30)
                nc.vector.memset(lrun[m][:], 0.0)
                nc.vector.memset(acc[m][:], 0.0)

            with (
                tc.tile_pool(name="spool", bufs=1) as spool,
                tc.tile_pool(name="sdram", bufs=1, space="DRAM") as sdram,
                tc.tile_pool(name="ps_m", bufs=3, space="PSUM") as ps_m,
                tc.tile_pool(name="ps_t", bufs=1, space="PSUM") as ps_t,
                tc.tile_pool(name="ps_xe", bufs=2, space="PSUM") as ps_xe,
                tc.tile_pool(name="ps_u", bufs=1, space="PSUM") as ps_u,
            ):
                vsT = [vspool.tile([P, MG], bf16, tag=f"vsT{ns}", name=f"vsT{ns}")
                       for ns in range(NSUB)]
                scache = [[sdram.tile([P, KS], bf16, tag=f"sc{s}_{ns}",
                                      name=f"sc{s}_{ns}")
                           for ns in range(NSUB)] for s in range(NSTR)]
                Sbuf = [[spool.tile([P, KS], bf16, tag=f"S{par}_{ns}",
                                    name=f"S{par}_{ns}")
                         for ns in range(NSUB)] for par in range(2)]

                # ---------- phase A: u = exp(relu(z)) @ [x|1] ----------
                # emitted first inside the big-loop scope: its PE work warms
                # the tensor engine and overlaps the S-strip prologue
                MB = 256
                for mb in range(MH // MB):           # 8 batches of 256 rows
                    ups = [ps_u.tile([P, F + 1], f32, tag=f"ups{q}", name=f"ups{q}")
                           for q in range(2)]
                    ez_pend = None
                    for ns in range(NSUB):
                        zps = ps_m.tile([P, KS], f32, tag="Mps", name="zps")
                        nc.tensor.matmul(zps[:, :MB], embT[:, ns * P:(ns + 1) * P],
                                         embhT[:, mb * MB:(mb + 1) * MB],
                                         start=True, stop=True)
                        ez = expp.tile([P, MB], bf16, tag="ez", name="ez")
                        nc.scalar.activation(ez[:], zps[:, :MB], AF.Exp)
                        # exp(relu(z)) = max(exp(z), 1)
                        nc.vector.tensor_scalar_max(ez[:], ez[:], 1.0)
                        if ez_pend is not None:
                            pez, pns = ez_pend
                            for q in range(2):
                                nc.tensor.matmul(ups[q][:], pez[:, q * P:(q + 1) * P],
                                                 xe_b[pns][:],
                                                 start=(pns == 0), stop=False)
                        ez_pend = (ez, ns)
                    pez, pns = ez_pend
                    for q in range(2):
                        nc.tensor.matmul(ups[q][:], pez[:, q * P:(q + 1) * P],
                                         xe_b[pns][:], start=False, stop=True)
                    for q in range(2):
                        nc.vector.tensor_copy(uacc[mb * 2 + q][:], ups[q][:])

                # fold xa into rest: rest += (0.5*sa*cw/rowsum)*u + 0.5*sa*cb
                for m in range(MH // P):
                    rcp = workp.tile([P, 1], f32, tag="rcp", name="rcp")
                    nc.vector.reciprocal(rcp[:], uacc[m][:, F:F + 1])
                    s1 = workp.tile([P, 1], f32, tag="s1", name="s1")
                    nc.vector.tensor_tensor(s1[:], sa[:, m:m + 1], rcp[:], op=ALU.mult)
                    nc.vector.tensor_scalar_mul(s1[:], s1[:], 0.5)
                    nc.vector.tensor_tensor(s1[:], s1[:], cvb[:, 0:1], op=ALU.mult)
                    s0 = workp.tile([P, 1], f32, tag="s0", name="s0")
                    nc.vector.tensor_tensor(s0[:], sa[:, m:m + 1], cvb[:, 1:2],
                                            op=ALU.mult)
                    nc.vector.tensor_scalar_mul(s0[:], s0[:], 0.5)
                    xat = workp.tile([P, F], f32, tag="xat", name="xat")
                    nc.vector.tensor_scalar(xat[:], uacc[m][:, :F], s1[:, 0:1],
                                            s0[:, 0:1], op0=ALU.mult, op1=ALU.add)
                    nc.vector.tensor_tensor(rest[m][:], rest[m][:], xat[:], op=ALU.add)

                def pe_tail(gm, et, r, s):
                    # transpose exp, xe partial, acc/l updates (PE + DVE tail)
                    etT = expp.tile([P, KS], bf16, tag="etT", name="etT")
                    for q in range(4):
                        tps = ps_t.tile([P, P], bf16, tag="tps", name="tps")
                        nc.tensor.transpose(tps[:], et[:, q * P:(q + 1) * P],
                                            ident[:])
                        nc.scalar.copy(etT[:, q * P:(q + 1) * P], tps[:])
                    xeps = ps_xe.tile([P, F], f32, tag="xeps", name="xeps")
                    for q in range(4):
                        nc.tensor.matmul(xeps[:], etT[:, q * P:(q + 1) * P],
                                         xe_b[s * 4 + q][:, :F],
                                         start=(q == 0), stop=(q == 3))
                    # acc = acc*r + xeps
                    nc.vector.tensor_scalar_mul(acc[gm][:], acc[gm][:], r[:, 0:1])
                    nc.vector.tensor_tensor(acc[gm][:], acc[gm][:], xeps[:],
                                            op=ALU.add)

                for g in range(NG):
                    pend = None
                    for ns in range(NSUB):
                        nc.sync.dma_start(
                            vsT[ns][:],
                            d_vsT[ns * P:(ns + 1) * P, g * MG:(g + 1) * MG])
                    for s in range(NSTR):
                        Scur = Sbuf[s % 2]
                        k0 = s * KS
                        # S' = tanh((e1*e2 + bs)/2) = 2*sigmoid(e1*e2+bs)-1
                        # (affine absorbed: softmax logits use scale=0.5 and
                        #  the 0.5*rowsum(vs) shift cancels in softmax)
                        if g == 0:
                            for ns in range(NSUB):
                                bsq = bsqp.tile([P, KS], f32, tag="bsq", name="bsq")
                                nc.sync.dma_start(
                                    bsq[:], d_bs[ns * P:(ns + 1) * P, k0:k0 + KS])
                                arg = workp.tile([P, KS], f32, tag="arg", name="arg")
                                # e1[n]*e2[k] (scale = per-partition e1)
                                nc.vector.tensor_scalar_mul(
                                    arg[:], e2b[:, k0:k0 + KS],
                                    e12T[:, 2 * ns:2 * ns + 1])
                                nc.vector.tensor_tensor(arg[:], arg[:], bsq[:],
                                                        op=ALU.add)
                                nc.scalar.activation(Scur[ns][:], arg[:], AF.Tanh,
                                                     scale=0.5)
                                nc.sync.dma_start(scache[s][ns][:], Scur[ns][:])
                        else:
                            for ns in range(NSUB):
                                nc.sync.dma_start(Scur[ns][:], scache[s][ns][:])
                        # consume: per m-chunk of this group
                        for mc in range(MCH):
                            gm = g * MCH + mc
                            Mps = ps_m.tile([P, KS], f32, tag="Mps", name="Mps")
                            for ns in range(NSUB):
                                nc.tensor.matmul(Mps[:],
                                                 vsT[ns][:, mc * P:(mc + 1) * P],
                                                 Scur[ns][:],
                                                 start=(ns == 0),
                                                 stop=(ns == NSUB - 1))
                            mx = workp.tile([P, 1], f32, tag="mx", name="mx")
                            nc.vector.reduce_max(mx[:], Mps[:],
                                                 axis=mybir.AxisListType.X)
                            nm = workp.tile([P, 1], f32, tag="nm", name="nm")
                            nc.vector.tensor_tensor(nm[:], mrun[gm][:], mx[:],
                                                    op=ALU.max)
                            dm = workp.tile([P, 1], f32, tag="dm", name="dm")
                            nc.vector.tensor_tensor(dm[:], mrun[gm][:], nm[:],
                                                    op=ALU.subtract)
                            r = workp.tile([P, 1], f32, tag="r", name="r")
                            nc.scalar.activation(r[:], dm[:], AF.Exp, scale=0.5)
                            nc.vector.tensor_copy(mrun[gm][:], nm[:])
                            nnm = workp.tile([P, 1], f32, tag="nnm", name="nnm")
                            nc.vector.tensor_scalar_mul(nnm[:], nm[:], -0.5)
                            et = expp.tile([P, KS], bf16, tag="et", name="et")
                            se = workp.tile([P, 1], f32, tag="se", name="se")
                            nc.scalar.activation(et[:], Mps[:], AF.Exp,
                                                 bias=nnm[:, 0:1], scale=0.5,
                                                 accum_out=se[:, 0:1])
                            # l = l*r + se
                            nc.vector.tensor_scalar_mul(lrun[gm][:], lrun[gm][:],
                                                        r[:, 0:1])
                            nc.vector.tensor_tensor(lrun[gm][:], lrun[gm][:], se[:],
                                                    op=ALU.add)
                            # flush previous chunk's PE tail (pipelined by one
                            # so the transposes never stall the PE stream)
                            if pend is not None:
                                pe_tail(*pend)
                            pend = (gm, et, r, s)
                    # flush last pending tail of this group
                    if pend is not None:
                        pe_tail(*pend)
                        pend = None
                    # epilogue for group g
                    for mc in range(MCH):
                        gm = g * MCH + mc
                        rl = workp.tile([P, 1], f32, tag="rl", name="rl")
                        nc.vector.reciprocal(rl[:], lrun[gm][:])
                        fin = workp.tile([P, F], f32, tag="fin", name="fin")
                        nc.vector.tensor_scalar_mul(fin[:], acc[gm][:], rl[:, 0:1])
                        nc.vector.tensor_tensor(fin[:], fin[:], rest[gm][:],
                                                op=ALU.add)
                        outt = workp.tile([P, F], f32, tag="outt", name="outt")
                        nc.scalar.activation(outt[:], fin[:], AF.Tanh)
                        nc.sync.dma_start(d_out[gm * P:(gm + 1) * P, :], outt[:])

    nc.compile()
    return nc


def _in_maps(x, x0, alpha, beta, w, d, w1, w2, vs, bs, node_emb, conv_w, conv_b):
    bfl = ml_dtypes.bfloat16
    maps = []
    embT = np.ascontiguousarray(node_emb.T).astype(bfl)
    w12 = np.ascontiguousarray(np.stack([w1, w2], axis=1))
    wT = np.ascontiguousarray(w.T)
    cv = np.array([[conv_w[0], conv_b[0]]], dtype=np.float32)
    for c in range(8):
        b, h = c // 2, c % 2
        rows = slice(h * MH, (h + 1) * MH)
        xb = x[b]
        maps.append({
            "x_bT": np.ascontiguousarray(xb.T),
            "x_b": np.ascontiguousarray(xb),
            "x_h": np.ascontiguousarray(xb[rows]),
            "x_hT": np.ascontiguousarray(xb.T[:, rows]),
            "x0_h": np.ascontiguousarray(x0[b, rows]),
            "alpha_h": np.ascontiguousarray(alpha[rows]),
            "beta_h": np.ascontiguousarray(beta[rows]),
            "w12": w12,
            "wT": wT,
            "d": np.ascontiguousarray(d),
            "conv2": cv,
            "vs_hT": np.ascontiguousarray(vs[rows].T).astype(bfl),
            "bs": np.ascontiguousarray(bs),
            "embT": embT,
            "emb_hT": np.ascontiguousarray(node_emb[rows].T).astype(bfl),
        })
    return maps


def kernel(**inputs):
    inputs = {k: np.asarray(v) for k, v in inputs.items()}
    x = inputs["x"].astype(np.float32)
    if "nc" not in _CACHE:
        _CACHE["nc"] = build_nc()
    nc = _CACHE["nc"]
    maps = _in_maps(
        x, inputs["x0"].astype(np.float32), inputs["alpha"].astype(np.float32),
        inputs["beta"].astype(np.float32), inputs["w"].astype(np.float32),
        inputs["d"].astype(np.float32), inputs["w1"].astype(np.float32),
        inputs["w2"].astype(np.float32), inputs["vs"].astype(np.float32),
        inputs["bs"].astype(np.float32), inputs["node_emb"].astype(np.float32),
        inputs["conv_w"].astype(np.float32), inputs["conv_b"].astype(np.float32))
    res = run_bass_kernel_spmd(nc, maps, core_ids=list(range(8)))
    out = np.empty((B, N, F), dtype=np.float32)
    for c in range(8):
        b, h = c // 2, c % 2
        out[b, h * MH:(h + 1) * MH] = np.asarray(res.results[c]["out"])
    return out


# revision 18
# speedup vs baseline: 1.1562x; 1.1562x over previous
"""Trainium2 Bass kernel for AGCNODEFunc (gnn_message_passing).

f = tanh(xe + 0.5*a*xa + x@W + x0*sig(beta) - 3x) where
  adj = softmax(relu(emb@emb.T), axis=1); xa = cw*(adj@x)+cb
  S[n,k] = sigmoid(e1[n]e2[k] + bs[n,k]); M = vs@S; Emat = softmax(M, -1); xe = Emat@x
  W = (w*clip(d,0,1))@w.T

Sharding: 8 cores = 4 batches x 2 row-halves. Core c: b=c//2, h=c%2, rows
[h*2048,(h+1)*2048). Fully data-parallel, no collectives. The N^3 matmul
(vs @ S) runs in bf16 on TensorE with flash-style online softmax over k
strips; m is processed in 2 groups of 1024 rows so vs_T(group) + double-
buffered S strips fit SBUF.
"""

import numpy as np
import ml_dtypes

import concourse.bass as bass
import concourse.bacc as bacc
import concourse.mybir as mybir
from concourse import tile, masks
from concourse.bass_utils import run_bass_kernel_spmd

B, N, F, E = 4, 4096, 64, 16
P = 128
MH = N // 2            # 2048 rows per core
NG = 2                 # m-groups per core
MG = MH // NG          # 1024 rows per group
MCH = MG // P          # 8 m-chunks per group
KS = 512               # k-strip width
NSTR = N // KS         # 8 strips per group sweep
NSUB = N // P          # 32 n-subtiles
XT = N // P            # 32 x tiles
f32 = mybir.dt.float32
bf16 = mybir.dt.bfloat16
AF = mybir.ActivationFunctionType
ALU = mybir.AluOpType

_CACHE = {}


def build_nc():
    nc = bacc.Bacc()
    # per-core DRAM parameters
    d_xbT = nc.dram_tensor("x_bT", (F, N), f32, kind="ExternalInput")
    d_xb = nc.dram_tensor("x_b", (N, F), f32, kind="ExternalInput")
    d_xh = nc.dram_tensor("x_h", (MH, F), f32, kind="ExternalInput")
    d_xhT = nc.dram_tensor("x_hT", (F, MH), f32, kind="ExternalInput")
    d_x0 = nc.dram_tensor("x0_h", (MH, F), f32, kind="ExternalInput")
    d_al = nc.dram_tensor("alpha_h", (MH,), f32, kind="ExternalInput")
    d_be = nc.dram_tensor("beta_h", (MH,), f32, kind="ExternalInput")
    d_w12 = nc.dram_tensor("w12", (F, 2), f32, kind="ExternalInput")
    d_wT = nc.dram_tensor("wT", (F, F), f32, kind="ExternalInput")
    d_d = nc.dram_tensor("d", (F,), f32, kind="ExternalInput")
    d_cv = nc.dram_tensor("conv2", (1, 2), f32, kind="ExternalInput")
    d_vsT = nc.dram_tensor("vs_hT", (N, MH), bf16, kind="ExternalInput")
    d_bs = nc.dram_tensor("bs", (N, N), f32, kind="ExternalInput")
    d_embT = nc.dram_tensor("embT", (E, N), bf16, kind="ExternalInput")
    d_embhT = nc.dram_tensor("emb_hT", (E, MH), bf16, kind="ExternalInput")
    d_out = nc.dram_tensor("out", (MH, F), f32, kind="ExternalOutput")

    with tile.TileContext(nc) as tc:
        with (
            tc.tile_pool(name="persist", bufs=1) as persist,
            tc.tile_pool(name="vspool", bufs=1) as vspool,
            tc.tile_pool(name="bsq", bufs=4) as bsqp,
            tc.tile_pool(name="work", bufs=4) as workp,
            tc.tile_pool(name="exp", bufs=3) as expp,
        ):
            ident = persist.tile([P, P], bf16)
            masks.make_identity(nc, ident[:])

            # persistent small tensors
            e2b = persist.tile([P, N], bf16)      # e2 broadcast over partitions
            e12T = persist.tile([P, 2 * NSUB], f32)
            sa = persist.tile([P, MH // P], f32)
            sb_ = persist.tile([P, MH // P], f32)
            cvb = persist.tile([P, 2], f32)
            xe_b = [persist.tile([P, F + 1], bf16, tag=f"xeb{k}", name=f"xeb{k}")
                    for k in range(XT)]
            rest = [persist.tile([P, F], f32, tag=f"rest{m}", name=f"rest{m}")
                    for m in range(MH // P)]
            uacc = [persist.tile([P, F + 1], f32, tag=f"u{m}", name=f"u{m}")
                    for m in range(MH // P)]
            mrun = [persist.tile([P, 1], f32, tag=f"mr{m}", name=f"mr{m}")
                    for m in range(MH // P)]
            lrun = [persist.tile([P, 1], f32, tag=f"lr{m}", name=f"lr{m}")
                    for m in range(MH // P)]
            acc = [persist.tile([P, F], f32, tag=f"acc{m}", name=f"acc{m}")
                   for m in range(MH // P)]
            embT = persist.tile([E, N], bf16)
            embhT = persist.tile([E, MH], bf16)
            nc.sync.dma_start(embT[:], d_embT[:])
            nc.sync.dma_start(embhT[:], d_embhT[:])

            with (
                tc.tile_pool(name="prep", bufs=1) as prep,
                tc.tile_pool(name="xrot", bufs=3) as xrot,
                tc.tile_pool(name="ps_prep", bufs=2, space="PSUM") as ps_prep,
            ):
                # ---------- prep: small parameter math ----------
                wt = prep.tile([F, F], f32)
                nc.sync.dma_start(wt[:], d_wT[:])
                dd = prep.tile([F, 1], f32)
                nc.sync.dma_start(dd[:], d_d[:].rearrange("(f o) -> f o", o=1))
                dcl = prep.tile([F, 1], f32)
                nc.scalar.activation(dcl[:], dd[:], AF.Relu)
                nc.vector.tensor_scalar_min(dcl[:], dcl[:], 1.0)
                wtd = prep.tile([F, F], f32)
                nc.scalar.mul(wtd[:], wt[:], dcl[:, 0:1])
                Wps = ps_prep.tile([P, KS], f32, tag="pp", name="Wps")
                nc.tensor.matmul(Wps[:F, :F], wtd[:], wt[:], start=True, stop=True)
                Wsb = prep.tile([F, F], f32)
                nc.vector.tensor_copy(Wsb[:], Wps[:F, :F])

                w12 = prep.tile([F, 2], f32)
                nc.sync.dma_start(w12[:], d_w12[:])
                xbt = prep.tile([F, N], f32)
                nc.sync.dma_start(xbt[:], d_xbT[:])
                xht = prep.tile([F, MH], f32)
                nc.sync.dma_start(xht[:], d_xhT[:])

                # e2 row form (1, N) on partition 0
                e2row = prep.tile([1, N], f32)
                for c in range(N // 512):
                    eps = ps_prep.tile([P, KS], f32, tag="pp", name="eps")
                    nc.tensor.matmul(eps[:1, :], w12[:, 1:2],
                                     xbt[:, c * 512:(c + 1) * 512],
                                     start=True, stop=True)
                    nc.vector.tensor_copy(e2row[:, c * 512:(c + 1) * 512], eps[:1, :])
                # e12 column form per n-subtile: (128, 2), col0 = e1
                for ns in range(NSUB):
                    eps2 = ps_prep.tile([P, KS], f32, tag="pp", name="eps2")
                    nc.tensor.matmul(eps2[:, :2], xbt[:, ns * P:(ns + 1) * P], w12[:],
                                     start=True, stop=True)
                    nc.vector.tensor_copy(e12T[:, 2 * ns:2 * ns + 2], eps2[:, :2])
                # e2 broadcast to all partitions, bf16
                e2bf = prep.tile([1, N], bf16)
                nc.vector.tensor_copy(e2bf[:], e2row[:])
                nc.gpsimd.partition_broadcast(e2b[:], e2bf[:])

                # alpha/beta sigmoids (128, 16)
                alp = prep.tile([P, MH // P], f32)
                nc.sync.dma_start(alp[:], d_al[:].rearrange("(c p) -> p c", p=P))
                nc.scalar.activation(sa[:], alp[:], AF.Sigmoid)
                bet = prep.tile([P, MH // P], f32)
                nc.sync.dma_start(bet[:], d_be[:].rearrange("(c p) -> p c", p=P))
                nc.scalar.activation(sb_[:], bet[:], AF.Sigmoid)
                cv1 = prep.tile([1, 2], f32)
                nc.sync.dma_start(cv1[:], d_cv[:])
                nc.gpsimd.partition_broadcast(cvb[:], cv1[:])

                # x tiles -> bf16 with ones column (persistent), f32 rotating
                for k in range(XT):
                    xfk = xrot.tile([P, F], f32, tag="xf", name="xf")
                    nc.sync.dma_start(xfk[:], d_xb[k * P:(k + 1) * P, :])
                    nc.scalar.copy(xe_b[k][:, :F], xfk[:])
                    nc.vector.memset(xe_b[k][:, F:F + 1], 1.0)

                # ---------- rest = xw + x0*sig(beta) - 3x ----------
                for m in range(MH // P):
                    xhm = xrot.tile([P, F], f32, tag="xh", name="xh")
                    nc.sync.dma_start(xhm[:], d_xh[m * P:(m + 1) * P, :])
                    x0m = xrot.tile([P, F], f32, tag="x0", name="x0")
                    nc.sync.dma_start(x0m[:], d_x0[m * P:(m + 1) * P, :])
                    xwps = ps_prep.tile([P, KS], f32, tag="pp", name="xwps")
                    nc.tensor.matmul(xwps[:, :F], xht[:, m * P:(m + 1) * P], Wsb[:],
                                     start=True, stop=True)
                    nc.vector.tensor_scalar_mul(rest[m][:], x0m[:], sb_[:, m:m + 1])
                    nc.vector.tensor_tensor(rest[m][:], rest[m][:], xwps[:, :F],
                                            op=ALU.add)
                    tmp3 = workp.tile([P, F], f32, tag="tmp3", name="tmp3")
                    nc.vector.tensor_scalar_mul(tmp3[:], xhm[:], -3.0)
                    nc.vector.tensor_tensor(rest[m][:], rest[m][:], tmp3[:], op=ALU.add)


            # ---------- big loop: M = vs@S, online softmax, xe ----------
            for m in range(MH // P):
                nc.vector.memset(mrun[m][:], -1e30)
                nc.vector.memset(lrun[m][:], 0.0)
                nc.vector.memset(acc[m][:], 0.0)

            with (
                tc.tile_pool(name="spool", bufs=1) as spool,
                tc.tile_pool(name="sdram", bufs=1, space="DRAM") as sdram,
                tc.tile_pool(name="ps_m", bufs=3, space="PSUM") as ps_m,
                tc.tile_pool(name="ps_t", bufs=1, space="PSUM") as ps_t,
                tc.tile_pool(name="ps_xe", bufs=2, space="PSUM") as ps_xe,
                tc.tile_pool(name="ps_u", bufs=1, space="PSUM") as ps_u,
            ):
                vsT = [vspool.tile([P, MG], bf16, tag=f"vsT{ns}", name=f"vsT{ns}")
                       for ns in range(NSUB)]
                scache = [[sdram.tile([P, KS], bf16, tag=f"sc{s}_{ns}",
                                      name=f"sc{s}_{ns}")
                           for ns in range(NSUB)] for s in range(NSTR)]
                Sbuf = [[spool.tile([P, KS], bf16, tag=f"S{par}_{ns}",
                                    name=f"S{par}_{ns}")
                         for ns in range(NSUB)] for par in range(2)]

                # ---------- phase A: u = exp(relu(z)) @ [x|1] ----------
                # emitted first inside the big-loop scope: its PE work warms
                # the tensor engine and overlaps the S-strip prologue.
                # u is accumulated TRANSPOSED (65, 512) so one psum bank covers
                # a whole 512-row batch; transposed back at the end.
                identf = persist.tile([P, P], f32, name="identf")
                masks.make_identity(nc, identf[:])
                uT = persist.tile([F + 1, MH], f32)
                MB = 512
                for mb in range(MH // MB):           # 4 batches of 512 rows
                    upsT = ps_u.tile([F + 1, MB], f32, tag="upsT", name="upsT")
                    for ns in range(NSUB):
                        zps = ps_m.tile([P, KS], f32, tag="Mps", name="zps")
                        nc.tensor.matmul(zps[:, :MB], embT[:, ns * P:(ns + 1) * P],
                                         embhT[:, mb * MB:(mb + 1) * MB],
                                         start=True, stop=True)
                        ez = expp.tile([P, MB], bf16, tag="ez", name="ez")
                        nc.scalar.activation(ez[:], zps[:, :MB], AF.Exp)
                        # exp(relu(z)) = max(exp(z), 1)
                        nc.vector.tensor_scalar_max(ez[:], ez[:], 1.0)
                        # u.T[f, m] += sum_j x_ext[j, f] * ez[j, m]
                        nc.tensor.matmul(upsT[:], xe_b[ns][:], ez[:],
                                         start=(ns == 0), stop=(ns == NSUB - 1))
                    nc.vector.tensor_copy(uT[:, mb * MB:(mb + 1) * MB], upsT[:])
                # transpose u.T -> uacc (m, 65)
                for m in range(MH // P):
                    tpu = ps_t.tile([P, P], f32, tag="tpu", name="tpu")
                    nc.tensor.transpose(tpu[:, :F + 1],
                                        uT[:, m * P:(m + 1) * P],
                                        identf[:F + 1, :F + 1])
                    nc.vector.tensor_copy(uacc[m][:], tpu[:, :F + 1])

                # fold xa into rest: rest += (0.5*sa*cw/rowsum)*u + 0.5*sa*cb
                for m in range(MH // P):
                    rcp = workp.tile([P, 1], f32, tag="rcp", name="rcp")
                    nc.vector.reciprocal(rcp[:], uacc[m][:, F:F + 1])
                    s1 = workp.tile([P, 1], f32, tag="s1", name="s1")
                    nc.vector.tensor_tensor(s1[:], sa[:, m:m + 1], rcp[:], op=ALU.mult)
                    nc.vector.tensor_scalar_mul(s1[:], s1[:], 0.5)
                    nc.vector.tensor_tensor(s1[:], s1[:], cvb[:, 0:1], op=ALU.mult)
                    s0 = workp.tile([P, 1], f32, tag="s0", name="s0")
                    nc.vector.tensor_tensor(s0[:], sa[:, m:m + 1], cvb[:, 1:2],
                                            op=ALU.mult)
                    nc.vector.tensor_scalar_mul(s0[:], s0[:], 0.5)
                    xat = workp.tile([P, F], f32, tag="xat", name="xat")
                    nc.vector.tensor_scalar(xat[:], uacc[m][:, :F], s1[:, 0:1],
                                            s0[:, 0:1], op0=ALU.mult, op1=ALU.add)
                    nc.vector.tensor_tensor(rest[m][:], rest[m][:], xat[:], op=ALU.add)

                for g in range(NG):
                    for ns in range(NSUB):
                        nc.sync.dma_start(
                            vsT[ns][:],
                            d_vsT[ns * P:(ns + 1) * P, g * MG:(g + 1) * MG])
                    for s in range(NSTR):
                        Scur = Sbuf[s % 2]
                        k0 = s * KS
                        # S' = tanh((e1*e2 + bs)/2) = 2*sigmoid(e1*e2+bs)-1
                        # (affine absorbed: softmax logits use scale=0.5 and
                        #  the 0.5*rowsum(vs) shift cancels in softmax)
                        if g == 0:
                            for ns in range(NSUB):
                                bsq = bsqp.tile([P, KS], f32, tag="bsq", name="bsq")
                                nc.sync.dma_start(
                                    bsq[:], d_bs[ns * P:(ns + 1) * P, k0:k0 + KS])
                                arg = workp.tile([P, KS], f32, tag="arg", name="arg")
                                # e1[n]*e2[k] (scale = per-partition e1)
                                nc.vector.tensor_scalar_mul(
                                    arg[:], e2b[:, k0:k0 + KS],
                                    e12T[:, 2 * ns:2 * ns + 1])
                                nc.vector.tensor_tensor(arg[:], arg[:], bsq[:],
                                                        op=ALU.add)
                                nc.scalar.activation(Scur[ns][:], arg[:], AF.Tanh,
                                                     scale=0.5)
                                nc.sync.dma_start(scache[s][ns][:], Scur[ns][:])
                        else:
                            for ns in range(NSUB):
                                nc.sync.dma_start(Scur[ns][:], scache[s][ns][:])
                        # consume: per m-chunk of this group
                        for mc in range(MCH):
                            gm = g * MCH + mc
                            Mps = ps_m.tile([P, KS], f32, tag="Mps", name="Mps")
                            for ns in range(NSUB):
                                nc.tensor.matmul(Mps[:],
                                                 vsT[ns][:, mc * P:(mc + 1) * P],
                                                 Scur[ns][:],
                                                 start=(ns == 0),
                                                 stop=(ns == NSUB - 1))
                            mx = workp.tile([P, 1], f32, tag="mx", name="mx")
                            nc.vector.reduce_max(mx[:], Mps[:],
                                                 axis=mybir.AxisListType.X)
                            nm = workp.tile([P, 1], f32, tag="nm", name="nm")
                            nc.vector.tensor_tensor(nm[:], mrun[gm][:], mx[:],
                                                    op=ALU.max)
                            dm = workp.tile([P, 1], f32, tag="dm", name="dm")
                            nc.vector.tensor_tensor(dm[:], mrun[gm][:], nm[:],
                                                    op=ALU.subtract)
                            r = workp.tile([P, 1], f32, tag="r", name="r")
                            nc.scalar.activation(r[:], dm[:], AF.Exp, scale=0.5)
                            nc.vector.tensor_copy(mrun[gm][:], nm[:])
                            nnm = workp.tile([P, 1], f32, tag="nnm", name="nnm")
                            nc.vector.tensor_scalar_mul(nnm[:], nm[:], -0.5)
                            et = expp.tile([P, KS], bf16, tag="et", name="et")
                            se = workp.tile([P, 1], f32, tag="se", name="se")
                            nc.scalar.activation(et[:], Mps[:], AF.Exp,
                                                 bias=nnm[:, 0:1], scale=0.5,
                                                 accum_out=se[:, 0:1])
                            # l = l*r + se
                            nc.vector.tensor_scalar_mul(lrun[gm][:], lrun[gm][:],
                                                        r[:, 0:1])
                            nc.vector.tensor_tensor(lrun[gm][:], lrun[gm][:], se[:],
                                                    op=ALU.add)
                            # transpose exp, xe partial
                            etT = expp.tile([P, KS], bf16, tag="etT", name="etT")
                            for q in range(4):
                                tps = ps_t.tile([P, P], bf16, tag="tps", name="tps")
                                nc.tensor.transpose(tps[:], et[:, q * P:(q + 1) * P],
                                                    ident[:])
                                nc.scalar.copy(etT[:, q * P:(q + 1) * P], tps[:])
                            xeps = ps_xe.tile([P, F], f32, tag="xeps", name="xeps")
                            for q in range(4):
                                nc.tensor.matmul(xeps[:], etT[:, q * P:(q + 1) * P],
                                                 xe_b[s * 4 + q][:, :F],
                                                 start=(q == 0), stop=(q == 3))
                            # acc = acc*r + xeps
                            nc.vector.tensor_scalar_mul(acc[gm][:], acc[gm][:],
                                                        r[:, 0:1])
                            nc.vector.tensor_tensor(acc[gm][:], acc[gm][:], xeps[:],
                                                    op=ALU.add)
                    # epilogue for group g
                    for mc in range(MCH):
                        gm = g * MCH + mc
                        rl = workp.tile([P, 1], f32, tag="rl", name="rl")
                        nc.vector.reciprocal(rl[:], lrun[gm][:])
                        fin = workp.tile([P, F], f32, tag="fin", name="fin")
                        nc.vector.tensor_scalar_mul(fin[:], acc[gm][:], rl[:, 0:1])
                        nc.vector.tensor_tensor(fin[:], fin[:], rest[gm][:],
                                                op=ALU.add)
                        outt = workp.tile([P, F], f32, tag="outt", name="outt")
                        nc.scalar.activation(outt[:], fin[:], AF.Tanh)
                        nc.sync.dma_start(d_out[gm * P:(gm + 1) * P, :], outt[:])

    nc.compile()
    return nc


def _in_maps(x, x0, alpha, beta, w, d, w1, w2, vs, bs, node_emb, conv_w, conv_b):
    bfl = ml_dtypes.bfloat16
    maps = []
    embT = np.ascontiguousarray(node_emb.T).astype(bfl)
    w12 = np.ascontiguousarray(np.stack([w1, w2], axis=1))
    wT = np.ascontiguousarray(w.T)
    cv = np.array([[conv_w[0], conv_b[0]]], dtype=np.float32)
    for c in range(8):
        b, h = c // 2, c % 2
        rows = slice(h * MH, (h + 1) * MH)
        xb = x[b]
        maps.append({
            "x_bT": np.ascontiguousarray(xb.T),
            "x_b": np.ascontiguousarray(xb),
            "x_h": np.ascontiguousarray(xb[rows]),
            "x_hT": np.ascontiguousarray(xb.T[:, rows]),
            "x0_h": np.ascontiguousarray(x0[b, rows]),
            "alpha_h": np.ascontiguousarray(alpha[rows]),
            "beta_h": np.ascontiguousarray(beta[rows]),
            "w12": w12,
            "wT": wT,
            "d": np.ascontiguousarray(d),
            "conv2": cv,
            "vs_hT": np.ascontiguousarray(vs[rows].T).astype(bfl),
            "bs": np.ascontiguousarray(bs),
            "embT": embT,
            "emb_hT": np.ascontiguousarray(node_emb[rows].T).astype(bfl),
        })
    return maps


def kernel(**inputs):
    inputs = {k: np.asarray(v) for k, v in inputs.items()}
    x = inputs["x"].astype(np.float32)
    if "nc" not in _CACHE:
        _CACHE["nc"] = build_nc()
    nc = _CACHE["nc"]
    maps = _in_maps(
        x, inputs["x0"].astype(np.float32), inputs["alpha"].astype(np.float32),
        inputs["beta"].astype(np.float32), inputs["w"].astype(np.float32),
        inputs["d"].astype(np.float32), inputs["w1"].astype(np.float32),
        inputs["w2"].astype(np.float32), inputs["vs"].astype(np.float32),
        inputs["bs"].astype(np.float32), inputs["node_emb"].astype(np.float32),
        inputs["conv_w"].astype(np.float32), inputs["conv_b"].astype(np.float32))
    res = run_bass_kernel_spmd(nc, maps, core_ids=list(range(8)))
    out = np.empty((B, N, F), dtype=np.float32)
    for c in range(8):
        b, h = c // 2, c % 2
        out[b, h * MH:(h + 1) * MH] = np.asarray(res.results[c]["out"])
    return out


# revision 19
# speedup vs baseline: 1.1602x; 1.0035x over previous
"""Trainium2 Bass kernel for AGCNODEFunc (gnn_message_passing).

f = tanh(xe + 0.5*a*xa + x@W + x0*sig(beta) - 3x) where
  adj = softmax(relu(emb@emb.T), axis=1); xa = cw*(adj@x)+cb
  S[n,k] = sigmoid(e1[n]e2[k] + bs[n,k]); M = vs@S; Emat = softmax(M, -1); xe = Emat@x
  W = (w*clip(d,0,1))@w.T

Sharding: 8 cores = 4 batches x 2 row-halves. Core c: b=c//2, h=c%2, rows
[h*2048,(h+1)*2048). Fully data-parallel, no collectives. The N^3 matmul
(vs @ S) runs in bf16 on TensorE with flash-style online softmax over k
strips; m is processed in 2 groups of 1024 rows so vs_T(group) + double-
buffered S strips fit SBUF.
"""

import numpy as np
import ml_dtypes

import concourse.bass as bass
import concourse.bacc as bacc
import concourse.mybir as mybir
from concourse import tile, masks
from concourse.bass_utils import run_bass_kernel_spmd

B, N, F, E = 4, 4096, 64, 16
P = 128
MH = N // 2            # 2048 rows per core
NG = 2                 # m-groups per core
MG = MH // NG          # 1024 rows per group
MCH = MG // P          # 8 m-chunks per group
KS = 512               # k-strip width
NSTR = N // KS         # 8 strips per group sweep
NSUB = N // P          # 32 n-subtiles
XT = N // P            # 32 x tiles
f32 = mybir.dt.float32
bf16 = mybir.dt.bfloat16
AF = mybir.ActivationFunctionType
ALU = mybir.AluOpType

_CACHE = {}


def build_nc():
    nc = bacc.Bacc()
    # per-core DRAM parameters
    d_xbT = nc.dram_tensor("x_bT", (F, N), f32, kind="ExternalInput")
    d_xb = nc.dram_tensor("x_b", (N, F), f32, kind="ExternalInput")
    d_xh = nc.dram_tensor("x_h", (MH, F), f32, kind="ExternalInput")
    d_xhT = nc.dram_tensor("x_hT", (F, MH), f32, kind="ExternalInput")
    d_x0 = nc.dram_tensor("x0_h", (MH, F), f32, kind="ExternalInput")
    d_al = nc.dram_tensor("alpha_h", (MH,), f32, kind="ExternalInput")
    d_be = nc.dram_tensor("beta_h", (MH,), f32, kind="ExternalInput")
    d_w12 = nc.dram_tensor("w12", (F, 2), f32, kind="ExternalInput")
    d_wT = nc.dram_tensor("wT", (F, F), f32, kind="ExternalInput")
    d_d = nc.dram_tensor("d", (F,), f32, kind="ExternalInput")
    d_cv = nc.dram_tensor("conv2", (1, 2), f32, kind="ExternalInput")
    d_vsT = nc.dram_tensor("vs_hT", (N, MH), bf16, kind="ExternalInput")
    d_bs = nc.dram_tensor("bs", (N, N), f32, kind="ExternalInput")
    d_embT = nc.dram_tensor("embT", (E, N), bf16, kind="ExternalInput")
    d_embhT = nc.dram_tensor("emb_hT", (E, MH), bf16, kind="ExternalInput")
    d_out = nc.dram_tensor("out", (MH, F), f32, kind="ExternalOutput")

    with tile.TileContext(nc) as tc:
        with (
            tc.tile_pool(name="persist", bufs=1) as persist,
            tc.tile_pool(name="vspool", bufs=1) as vspool,
            tc.tile_pool(name="bsq", bufs=4) as bsqp,
            tc.tile_pool(name="work", bufs=4) as workp,
            tc.tile_pool(name="exp", bufs=3) as expp,
        ):
            ident = persist.tile([P, P], bf16)
            masks.make_identity(nc, ident[:])

            # persistent small tensors
            e2b = persist.tile([P, N], bf16)      # e2 broadcast over partitions
            e12T = persist.tile([P, 2 * NSUB], f32)
            sa = persist.tile([P, MH // P], f32)
            sb_ = persist.tile([P, MH // P], f32)
            cvb = persist.tile([P, 2], f32)
            xe_b = [persist.tile([P, F + 1], bf16, tag=f"xeb{k}", name=f"xeb{k}")
                    for k in range(XT)]
            rest = [persist.tile([P, F], f32, tag=f"rest{m}", name=f"rest{m}")
                    for m in range(MH // P)]
            uacc = [persist.tile([P, F + 1], f32, tag=f"u{m}", name=f"u{m}")
                    for m in range(MH // P)]
            mrun = [persist.tile([P, 1], f32, tag=f"mr{m}", name=f"mr{m}")
                    for m in range(MH // P)]
            lrun = [persist.tile([P, 1], f32, tag=f"lr{m}", name=f"lr{m}")
                    for m in range(MH // P)]
            acc = [persist.tile([P, F], f32, tag=f"acc{m}", name=f"acc{m}")
                   for m in range(MH // P)]
            embT = persist.tile([E, N], bf16)
            embhT = persist.tile([E, MH], bf16)
            nc.sync.dma_start(embT[:], d_embT[:])
            nc.sync.dma_start(embhT[:], d_embhT[:])

            with (
                tc.tile_pool(name="prep", bufs=1) as prep,
                tc.tile_pool(name="xrot", bufs=3) as xrot,
                tc.tile_pool(name="ps_prep", bufs=2, space="PSUM") as ps_prep,
            ):
                # ---------- prep: small parameter math ----------
                wt = prep.tile([F, F], f32)
                nc.sync.dma_start(wt[:], d_wT[:])
                dd = prep.tile([F, 1], f32)
                nc.sync.dma_start(dd[:], d_d[:].rearrange("(f o) -> f o", o=1))
                dcl = prep.tile([F, 1], f32)
                nc.scalar.activation(dcl[:], dd[:], AF.Relu)
                nc.vector.tensor_scalar_min(dcl[:], dcl[:], 1.0)
                wtd = prep.tile([F, F], f32)
                nc.scalar.mul(wtd[:], wt[:], dcl[:, 0:1])
                Wps = ps_prep.tile([P, KS], f32, tag="pp", name="Wps")
                nc.tensor.matmul(Wps[:F, :F], wtd[:], wt[:], start=True, stop=True)
                Wsb = prep.tile([F, F], f32)
                nc.vector.tensor_copy(Wsb[:], Wps[:F, :F])

                w12 = prep.tile([F, 2], f32)
                nc.sync.dma_start(w12[:], d_w12[:])
                xbt = prep.tile([F, N], f32)
                nc.sync.dma_start(xbt[:], d_xbT[:])
                xht = prep.tile([F, MH], f32)
                nc.sync.dma_start(xht[:], d_xhT[:])

                # e2 row form (1, N) on partition 0
                e2row = prep.tile([1, N], f32)
                for c in range(N // 512):
                    eps = ps_prep.tile([P, KS], f32, tag="pp", name="eps")
                    nc.tensor.matmul(eps[:1, :], w12[:, 1:2],
                                     xbt[:, c * 512:(c + 1) * 512],
                                     start=True, stop=True)
                    nc.vector.tensor_copy(e2row[:, c * 512:(c + 1) * 512], eps[:1, :])
                # e12 column form per n-subtile: (128, 2), col0 = e1
                for ns in range(NSUB):
                    eps2 = ps_prep.tile([P, KS], f32, tag="pp", name="eps2")
                    nc.tensor.matmul(eps2[:, :2], xbt[:, ns * P:(ns + 1) * P], w12[:],
                                     start=True, stop=True)
                    nc.vector.tensor_copy(e12T[:, 2 * ns:2 * ns + 2], eps2[:, :2])
                # e2 broadcast to all partitions, bf16
                e2bf = prep.tile([1, N], bf16)
                nc.vector.tensor_copy(e2bf[:], e2row[:])
                nc.gpsimd.partition_broadcast(e2b[:], e2bf[:])

                # alpha/beta sigmoids (128, 16)
                alp = prep.tile([P, MH // P], f32)
                nc.sync.dma_start(alp[:], d_al[:].rearrange("(c p) -> p c", p=P))
                nc.scalar.activation(sa[:], alp[:], AF.Sigmoid)
                bet = prep.tile([P, MH // P], f32)
                nc.sync.dma_start(bet[:], d_be[:].rearrange("(c p) -> p c", p=P))
                nc.scalar.activation(sb_[:], bet[:], AF.Sigmoid)
                cv1 = prep.tile([1, 2], f32)
                nc.sync.dma_start(cv1[:], d_cv[:])
                nc.gpsimd.partition_broadcast(cvb[:], cv1[:])

                # x tiles -> bf16 with ones column (persistent), f32 rotating
                for k in range(XT):
                    xfk = xrot.tile([P, F], f32, tag="xf", name="xf")
                    nc.sync.dma_start(xfk[:], d_xb[k * P:(k + 1) * P, :])
                    nc.scalar.copy(xe_b[k][:, :F], xfk[:])
                    nc.vector.memset(xe_b[k][:, F:F + 1], 1.0)

                # ---------- rest = xw + x0*sig(beta) - 3x ----------
                for m in range(MH // P):
                    xhm = xrot.tile([P, F], f32, tag="xh", name="xh")
                    nc.sync.dma_start(xhm[:], d_xh[m * P:(m + 1) * P, :])
                    x0m = xrot.tile([P, F], f32, tag="x0", name="x0")
                    nc.sync.dma_start(x0m[:], d_x0[m * P:(m + 1) * P, :])
                    xwps = ps_prep.tile([P, KS], f32, tag="pp", name="xwps")
                    nc.tensor.matmul(xwps[:, :F], xht[:, m * P:(m + 1) * P], Wsb[:],
                                     start=True, stop=True)
                    nc.vector.tensor_scalar_mul(rest[m][:], x0m[:], sb_[:, m:m + 1])
                    nc.vector.tensor_tensor(rest[m][:], rest[m][:], xwps[:, :F],
                                            op=ALU.add)
                    tmp3 = workp.tile([P, F], f32, tag="tmp3", name="tmp3")
                    nc.vector.tensor_scalar_mul(tmp3[:], xhm[:], -3.0)
                    nc.vector.tensor_tensor(rest[m][:], rest[m][:], tmp3[:], op=ALU.add)


            # ---------- big loop: M = vs@S, online softmax, xe ----------
            for m in range(MH // P):
                nc.vector.memset(mrun[m][:], -1e30)
                nc.vector.memset(lrun[m][:], 0.0)
                nc.vector.memset(acc[m][:], 0.0)

            with (
                tc.tile_pool(name="spool", bufs=1) as spool,
                tc.tile_pool(name="sdram", bufs=1, space="DRAM") as sdram,
                tc.tile_pool(name="ps_m", bufs=3, space="PSUM") as ps_m,
                tc.tile_pool(name="ps_t", bufs=1, space="PSUM") as ps_t,
                tc.tile_pool(name="ps_xe", bufs=2, space="PSUM") as ps_xe,
                tc.tile_pool(name="ps_u", bufs=1, space="PSUM") as ps_u,
            ):
                vsT = [vspool.tile([P, MG], bf16, tag=f"vsT{ns}", name=f"vsT{ns}")
                       for ns in range(NSUB)]
                scache = [[sdram.tile([P, KS], bf16, tag=f"sc{s}_{ns}",
                                      name=f"sc{s}_{ns}")
                           for ns in range(NSUB)] for s in range(NSTR)]
                Sbuf = [[spool.tile([P, KS], bf16, tag=f"S{par}_{ns}",
                                    name=f"S{par}_{ns}")
                         for ns in range(NSUB)] for par in range(2)]

                # ---------- phase A: u = exp(relu(z)) @ [x|1] ----------
                # emitted first inside the big-loop scope: its PE work warms
                # the tensor engine and overlaps the S-strip prologue.
                # u is accumulated TRANSPOSED (65, 512) so one psum bank covers
                # a whole 512-row batch; transposed back at the end.
                identf = persist.tile([P, P], f32, name="identf")
                masks.make_identity(nc, identf[:])
                uT = persist.tile([F + 1, MH], f32)
                MB = 512
                for mb in range(MH // MB):           # 4 batches of 512 rows
                    upsT = ps_u.tile([F + 1, MB], f32, tag="upsT", name="upsT")
                    for ns in range(NSUB):
                        zps = ps_m.tile([P, KS], f32, tag="Mps", name="zps")
                        nc.tensor.matmul(zps[:, :MB], embT[:, ns * P:(ns + 1) * P],
                                         embhT[:, mb * MB:(mb + 1) * MB],
                                         start=True, stop=True)
                        ez = expp.tile([P, MB], bf16, tag="ez", name="ez")
                        nc.scalar.activation(ez[:], zps[:, :MB], AF.Exp)
                        # exp(relu(z)) = max(exp(z), 1)
                        nc.vector.tensor_scalar_max(ez[:], ez[:], 1.0)
                        # u.T[f, m] += sum_j x_ext[j, f] * ez[j, m]
                        nc.tensor.matmul(upsT[:], xe_b[ns][:], ez[:],
                                         start=(ns == 0), stop=(ns == NSUB - 1))
                    nc.vector.tensor_copy(uT[:, mb * MB:(mb + 1) * MB], upsT[:])
                # transpose u.T -> uacc (m, 65)
                for m in range(MH // P):
                    tpu = ps_t.tile([P, P], f32, tag="tpu", name="tpu")
                    nc.tensor.transpose(tpu[:, :F + 1],
                                        uT[:, m * P:(m + 1) * P],
                                        identf[:F + 1, :F + 1])
                    nc.vector.tensor_copy(uacc[m][:], tpu[:, :F + 1])

                # fold xa into rest: rest += (0.5*sa*cw/rowsum)*u + 0.5*sa*cb
                for m in range(MH // P):
                    rcp = workp.tile([P, 1], f32, tag="rcp", name="rcp")
                    nc.vector.reciprocal(rcp[:], uacc[m][:, F:F + 1])
                    s1 = workp.tile([P, 1], f32, tag="s1", name="s1")
                    nc.vector.tensor_tensor(s1[:], sa[:, m:m + 1], rcp[:], op=ALU.mult)
                    nc.vector.tensor_scalar_mul(s1[:], s1[:], 0.5)
                    nc.vector.tensor_tensor(s1[:], s1[:], cvb[:, 0:1], op=ALU.mult)
                    s0 = workp.tile([P, 1], f32, tag="s0", name="s0")
                    nc.vector.tensor_tensor(s0[:], sa[:, m:m + 1], cvb[:, 1:2],
                                            op=ALU.mult)
                    nc.vector.tensor_scalar_mul(s0[:], s0[:], 0.5)
                    xat = workp.tile([P, F], f32, tag="xat", name="xat")
                    nc.vector.tensor_scalar(xat[:], uacc[m][:, :F], s1[:, 0:1],
                                            s0[:, 0:1], op0=ALU.mult, op1=ALU.add)
                    nc.vector.tensor_tensor(rest[m][:], rest[m][:], xat[:], op=ALU.add)

                pend = None
                for g in range(NG):
                    for ns in range(NSUB):
                        nc.sync.dma_start(
                            vsT[ns][:],
                            d_vsT[ns * P:(ns + 1) * P, g * MG:(g + 1) * MG])
                    for s in range(NSTR):
                        Scur = Sbuf[s % 2]
                        k0 = s * KS
                        # S' = tanh((e1*e2 + bs)/2) = 2*sigmoid(e1*e2+bs)-1
                        # (affine absorbed: softmax logits use scale=0.5 and
                        #  the 0.5*rowsum(vs) shift cancels in softmax)
                        if g == 0:
                            for ns in range(NSUB):
                                bsq = bsqp.tile([P, KS], f32, tag="bsq", name="bsq")
                                nc.sync.dma_start(
                                    bsq[:], d_bs[ns * P:(ns + 1) * P, k0:k0 + KS])
                                arg = workp.tile([P, KS], f32, tag="arg", name="arg")
                                # e1[n]*e2[k] (scale = per-partition e1)
                                nc.vector.tensor_scalar_mul(
                                    arg[:], e2b[:, k0:k0 + KS],
                                    e12T[:, 2 * ns:2 * ns + 1])
                                nc.vector.tensor_tensor(arg[:], arg[:], bsq[:],
                                                        op=ALU.add)
                                nc.scalar.activation(Scur[ns][:], arg[:], AF.Tanh,
                                                     scale=0.5)
                                nc.sync.dma_start(scache[s][ns][:], Scur[ns][:])
                        else:
                            for ns in range(NSUB):
                                nc.sync.dma_start(Scur[ns][:], scache[s][ns][:])
                        # consume: per m-chunk of this group.
                        # PE stream interleave: [16 MMs(i)] [tail(i-1)] [16 MMs(i)]
                        # so the transposes never stall the in-order PE queue.
                        for mc in range(MCH):
                            gm = g * MCH + mc
                            Mps = ps_m.tile([P, KS], f32, tag="Mps", name="Mps")
                            for ns in range(16):
                                nc.tensor.matmul(Mps[:],
                                                 vsT[ns][:, mc * P:(mc + 1) * P],
                                                 Scur[ns][:],
                                                 start=(ns == 0), stop=False)
                            if pend is not None:
                                pgm, pet, pr, ps_ = pend
                                etT = expp.tile([P, KS], bf16, tag="etT", name="etT")
                                for q in range(4):
                                    tps = ps_t.tile([P, P], bf16, tag="tps",
                                                    name="tps")
                                    nc.tensor.transpose(
                                        tps[:], pet[:, q * P:(q + 1) * P], ident[:])
                                    nc.scalar.copy(etT[:, q * P:(q + 1) * P], tps[:])
                                xeps = ps_xe.tile([P, F], f32, tag="xeps",
                                                  name="xeps")
                                for q in range(4):
                                    nc.tensor.matmul(xeps[:],
                                                     etT[:, q * P:(q + 1) * P],
                                                     xe_b[ps_ * 4 + q][:, :F],
                                                     start=(q == 0), stop=(q == 3))
                                nc.vector.tensor_scalar_mul(acc[pgm][:], acc[pgm][:],
                                                            pr[:, 0:1])
                                nc.vector.tensor_tensor(acc[pgm][:], acc[pgm][:],
                                                        xeps[:], op=ALU.add)
                                pend = None
                            for ns in range(16, NSUB):
                                nc.tensor.matmul(Mps[:],
                                                 vsT[ns][:, mc * P:(mc + 1) * P],
                                                 Scur[ns][:],
                                                 start=False,
                                                 stop=(ns == NSUB - 1))
                            mx = workp.tile([P, 1], f32, tag="mx", name="mx")
                            nc.vector.reduce_max(mx[:], Mps[:],
                                                 axis=mybir.AxisListType.X)
                            nm = workp.tile([P, 1], f32, tag="nm", name="nm")
                            nc.vector.tensor_tensor(nm[:], mrun[gm][:], mx[:],
                                                    op=ALU.max)
                            dm = workp.tile([P, 1], f32, tag="dm", name="dm")
                            nc.vector.tensor_tensor(dm[:], mrun[gm][:], nm[:],
                                                    op=ALU.subtract)
                            r = workp.tile([P, 1], f32, tag="r", name="r")
                            nc.scalar.activation(r[:], dm[:], AF.Exp, scale=0.5)
                            nc.vector.tensor_copy(mrun[gm][:], nm[:])
                            nnm = workp.tile([P, 1], f32, tag="nnm", name="nnm")
                            nc.vector.tensor_scalar_mul(nnm[:], nm[:], -0.5)
                            et = expp.tile([P, KS], bf16, tag="et", name="et")
                            se = workp.tile([P, 1], f32, tag="se", name="se")
                            nc.scalar.activation(et[:], Mps[:], AF.Exp,
                                                 bias=nnm[:, 0:1], scale=0.5,
                                                 accum_out=se[:, 0:1])
                            # l = l*r + se
                            nc.vector.tensor_scalar_mul(lrun[gm][:], lrun[gm][:],
                                                        r[:, 0:1])
                            nc.vector.tensor_tensor(lrun[gm][:], lrun[gm][:], se[:],
                                                    op=ALU.add)
                            pend = (gm, et, r, s)
                    # flush last pending tail before the group epilogue
                    if pend is not None:
                        pgm, pet, pr, ps_ = pend
                        etT = expp.tile([P, KS], bf16, tag="etT", name="etT")
                        for q in range(4):
                            tps = ps_t.tile([P, P], bf16, tag="tps", name="tps")
                            nc.tensor.transpose(tps[:], pet[:, q * P:(q + 1) * P],
                                                ident[:])
                            nc.scalar.copy(etT[:, q * P:(q + 1) * P], tps[:])
                        xeps = ps_xe.tile([P, F], f32, tag="xeps", name="xeps")
                        for q in range(4):
                            nc.tensor.matmul(xeps[:], etT[:, q * P:(q + 1) * P],
                                             xe_b[ps_ * 4 + q][:, :F],
                                             start=(q == 0), stop=(q == 3))
                        nc.vector.tensor_scalar_mul(acc[pgm][:], acc[pgm][:],
                                                    pr[:, 0:1])
                        nc.vector.tensor_tensor(acc[pgm][:], acc[pgm][:], xeps[:],
                                                op=ALU.add)
                        pend = None
                    # epilogue for group g
                    for mc in range(MCH):
                        gm = g * MCH + mc
                        rl = workp.tile([P, 1], f32, tag="rl", name="rl")
                        nc.vector.reciprocal(rl[:], lrun[gm][:])
                        fin = workp.tile([P, F], f32, tag="fin", name="fin")
                        nc.vector.tensor_scalar_mul(fin[:], acc[gm][:], rl[:, 0:1])
                        nc.vector.tensor_tensor(fin[:], fin[:], rest[gm][:],
                                                op=ALU.add)
                        outt = workp.tile([P, F], f32, tag="outt", name="outt")
                        nc.scalar.activation(outt[:], fin[:], AF.Tanh)
                        nc.sync.dma_start(d_out[gm * P:(gm + 1) * P, :], outt[:])

    nc.compile()
    return nc


def _in_maps(x, x0, alpha, beta, w, d, w1, w2, vs, bs, node_emb, conv_w, conv_b):
    bfl = ml_dtypes.bfloat16
    maps = []
    embT = np.ascontiguousarray(node_emb.T).astype(bfl)
    w12 = np.ascontiguousarray(np.stack([w1, w2], axis=1))
    wT = np.ascontiguousarray(w.T)
    cv = np.array([[conv_w[0], conv_b[0]]], dtype=np.float32)
    for c in range(8):
        b, h = c // 2, c % 2
        rows = slice(h * MH, (h + 1) * MH)
        xb = x[b]
        maps.append({
            "x_bT": np.ascontiguousarray(xb.T),
            "x_b": np.ascontiguousarray(xb),
            "x_h": np.ascontiguousarray(xb[rows]),
            "x_hT": np.ascontiguousarray(xb.T[:, rows]),
            "x0_h": np.ascontiguousarray(x0[b, rows]),
            "alpha_h": np.ascontiguousarray(alpha[rows]),
            "beta_h": np.ascontiguousarray(beta[rows]),
            "w12": w12,
            "wT": wT,
            "d": np.ascontiguousarray(d),
            "conv2": cv,
            "vs_hT": np.ascontiguousarray(vs[rows].T).astype(bfl),
            "bs": np.ascontiguousarray(bs),
            "embT": embT,
            "emb_hT": np.ascontiguousarray(node_emb[rows].T).astype(bfl),
        })
    return maps


def kernel(**inputs):
    inputs = {k: np.asarray(v) for k, v in inputs.items()}
    x = inputs["x"].astype(np.float32)
    if "nc" not in _CACHE:
        _CACHE["nc"] = build_nc()
    nc = _CACHE["nc"]
    maps = _in_maps(
        x, inputs["x0"].astype(np.float32), inputs["alpha"].astype(np.float32),
        inputs["beta"].astype(np.float32), inputs["w"].astype(np.float32),
        inputs["d"].astype(np.float32), inputs["w1"].astype(np.float32),
        inputs["w2"].astype(np.float32), inputs["vs"].astype(np.float32),
        inputs["bs"].astype(np.float32), inputs["node_emb"].astype(np.float32),
        inputs["conv_w"].astype(np.float32), inputs["conv_b"].astype(np.float32))
    res = run_bass_kernel_spmd(nc, maps, core_ids=list(range(8)))
    out = np.empty((B, N, F), dtype=np.float32)
    for c in range(8):
        b, h = c // 2, c % 2
        out[b, h * MH:(h + 1) * MH] = np.asarray(res.results[c]["out"])
    return out
